# revision 8
# baseline (speedup 1.0000x reference)
"""GAT (3 GAT layers + BN/ReLU + MLP head) on 8 Trainium2 NeuronCores.

Sharding: nodes split into 8 contiguous dst-shards (6250/core, padded to
6272). Edges (self-loops excluded; handled densely) are owned by the dst
core, sorted by dst block; per-block chunk counts are maxed across cores so
one SPMD program serves all cores. Per layer: local dense transform
(z @ [W | W@a_src | W@a_dst]) -> bf16 pack [h | asrc] AllGathered; a_dst
stays core-local in f32. Edge aggregation per 128-node dst block:
128-row indirect gathers, iota/is_equal one-hots, a_dst expanded to edges
via (one-hot * stride0-broadcast row) + free-axis reduce,
p = exp(leakyrelu(asrc + adst)), and accumulating one-hot scatter matmuls
into PSUM producing [nodes, H*D | H] (numerator | denominator).
BatchNorm stats via ones-vector matmuls + AllReduce (biases cancel in BN).
"""

import numpy as np
import ml_dtypes

N_CORES = 8
N = 50000
F_IN = 128
HID = 64
HEADS = 4
EPS_BN = 1e-5
NEG_SLOPE = 0.2
P = 128

_STATE = {}
PACK_F32 = False
DEBUG_DUMP = False


def _dims():
    NL = N // N_CORES
    NB = (NL + P - 1) // P
    NLP = NB * P
    LAST = NL - (NB - 1) * P
    return NL, NB, NLP, LAST


def _host_prep_edges(edge_index):
    NL, NB, NLP, LAST = _dims()
    src = np.asarray(edge_index[0], dtype=np.int64)
    dst = np.asarray(edge_index[1], dtype=np.int64)
    order = np.argsort(dst, kind="stable")
    src, dst = src[order], dst[order]

    core_of = dst // NL
    per_core = []
    counts = []
    for c in range(N_CORES):
        m = core_of == c
        s, d = src[m], dst[m]
        dl = d - c * NL
        bc = np.bincount(dl // P, minlength=NB)
        per_core.append((s, dl, bc))
        counts.append((bc + P - 1) // P)
    nch = np.maximum.reduce(counts)
    TOT = int(nch.sum())

    idx_bufs, dl_bufs = [], []
    for c in range(N_CORES):
        s, dl, bc = per_core[c]
        rows = (s // NL) * NLP + (s % NL)
        ib = np.zeros(TOT * P, np.int32)
        db = np.full(TOT * P, -1.0, np.float32)
        t0 = 0
        e0 = 0
        for b in range(NB):
            ne = int(bc[b])
            nchb = int(nch[b])
            r = rows[e0:e0 + ne]
            dloc = (dl[e0:e0 + ne] - b * P).astype(np.float32)
            pad = nchb * P - ne
            if pad:
                r = np.concatenate([r, np.zeros(pad, np.int64)])
                dloc = np.concatenate([dloc, np.full(pad, -1.0, np.float32)])
            # [nchb, P] -> transposed [P, nchb], flattened C-order
            ib[t0 * P:(t0 + nchb) * P] = (
                r.reshape(nchb, P).T.astype(np.int32).reshape(-1))
            db[t0 * P:(t0 + nchb) * P] = dloc.reshape(nchb, P).T.reshape(-1)
            e0 += ne
            t0 += nchb
        idx_bufs.append(ib)
        dl_bufs.append(db)
    return nch.astype(np.int64), idx_bufs, dl_bufs


def _build(nch):
    import concourse.bass as bass
    import concourse.bacc as bacc
    import concourse.mybir as mybir
    import concourse.tile as tile
    from concourse.masks import make_identity

    NL, NB, NLP, LAST = _dims()
    F32 = mybir.dt.float32
    BF = mybir.dt.float32 if PACK_F32 else mybir.dt.bfloat16
    I32 = mybir.dt.int32
    TOT = int(nch.sum())
    NCHM = int(nch.max())
    NF = N_CORES * NLP
    DH = HID * HEADS                       # 256, width of layer-0 state
    LAY = [(F_IN, HID, HEADS), (HID * HEADS, HID, 1), (HID, HID, 1)]

    nc = bacc.Bacc("TRN2", target_bir_lowering=False, debug=False,
                   num_devices=N_CORES)

    t_x = nc.dram_tensor("x", [NLP, F_IN], F32, kind="ExternalInput")
    t_rhs = [nc.dram_tensor(f"rhs{li}", [Din, D * H + 2 * H], F32,
                            kind="ExternalInput")
             for li, (Din, D, H) in enumerate(LAY)]
    t_lw1 = nc.dram_tensor("lw1", [HID, HID], F32, kind="ExternalInput")
    t_lw2 = nc.dram_tensor("lw2", [HID, 2], F32, kind="ExternalInput")
    t_lb2 = nc.dram_tensor("lb2", [1, 2], F32, kind="ExternalInput")
    t_iota = nc.dram_tensor("iota", [P, P], F32, kind="ExternalInput")
    t_idx = nc.dram_tensor("eidx", [TOT * P], I32, kind="ExternalInput")
    t_dl = nc.dram_tensor("edl", [TOT * P], F32, kind="ExternalInput")
    t_out = nc.dram_tensor("out", [NLP, 2], F32, kind="ExternalOutput")
    t_dbg = []
    if DEBUG_DUMP:
        for li in range(3):
            D = LAY[li][1] * LAY[li][2]
            t_dbg.append(nc.dram_tensor(f"dbg{li}", [NLP, D + HEADS], F32,
                                        kind="ExternalOutput"))

    with tile.TileContext(nc) as tc:
        with tc.tile_pool(name="cb", bufs=1) as cb, \
             tc.tile_pool(name="sp", bufs=2) as sp, \
             tc.tile_pool(name="spv", bufs=1) as spv, \
             tc.tile_pool(name="ps", bufs=2, space="PSUM") as ps, \
             tc.tile_pool(name="pst", bufs=2, space="PSUM") as pst, \
             tc.tile_pool(name="pss", bufs=1, space="PSUM") as pss, \
             tc.tile_pool(name="dr", bufs=1, space="DRAM") as dr:

            ident = cb.tile([P, P], F32)
            make_identity(nc, ident[:])
            iota_t = cb.tile([P, P], F32)
            nc.sync.dma_start(out=iota_t[:], in_=t_iota[:])
            ones_f = cb.tile([P, 1], F32)
            nc.vector.memset(ones_f[:], 1.0)
            ones_p = cb.tile([P, 1], F32)
            nc.vector.memset(ones_p[:], 0.0)
            nc.vector.memset(ones_p[:LAST], 1.0)
            lb2_b = cb.tile([P, 2], F32)
            nc.sync.dma_start(out=lb2_b[:],
                              in_=t_lb2[0, None, :].to_broadcast([P, 2]))

            ag_ins, ag_outs, adst_ds = [], [], []
            for li, (Din, D, H) in enumerate(LAY):
                R = D * H + H
                ag_ins.append(dr.tile([NLP, R], BF, name=f"agin{li}"))
                ag_outs.append(dr.tile([NF, R], BF, addr_space="Shared",
                                       name=f"agout{li}"))
                adst_ds.append(dr.tile([H, NLP], F32, name=f"adst{li}"))

            zcur = cb.tile([P, NB, DH], F32)      # state (aggregated / input)
            oden = cb.tile([P, NB, HEADS], F32)
            sc_t = cb.tile([P, DH], F32)          # BN scale (bcast)
            sh_t = cb.tile([P, DH], F32)          # BN shift (bcast)

            nc.sync.dma_start(out=zcur[:, :, :F_IN],
                              in_=t_x[:].rearrange("(b p) f -> p b f", p=P))

            def dense_phase(li, norm, Dprev):
                """pack_l = act(zcur) @ [W|was|wad] -> ag_in, adst; AllGather."""
                Din, D, H = LAY[li]
                R = D * H + H
                W = D * H + 2 * H
                nA = max(Din // P, 1)
                rhs_t = cb.tile([P, nA, W], F32, name=f"rhsl{li}")
                if Din >= P:
                    nc.sync.dma_start(
                        out=rhs_t[:],
                        in_=t_rhs[li][:].rearrange("(a p) w -> p a w", p=P))
                else:
                    nc.sync.dma_start(out=rhs_t[:Din, 0, :], in_=t_rhs[li][:])
                for b in range(NB):
                    if norm:
                        z_t = sp.tile([P, Dprev], F32, name="zt", tag="zt",
                                      padded_shape=[P, DH])
                        nc.vector.tensor_tensor(
                            out=z_t[:], in0=zcur[:, b, :Dprev],
                            in1=sc_t[:, :Dprev], op=mybir.AluOpType.mult)
                        nc.vector.tensor_tensor(
                            out=z_t[:], in0=z_t[:], in1=sh_t[:, :Dprev],
                            op=mybir.AluOpType.add)
                        nc.vector.tensor_scalar(
                            out=z_t[:], in0=z_t[:], scalar1=0.0, scalar2=None,
                            op0=mybir.AluOpType.max)
                        zsrc = z_t
                        zoff = 0
                    else:
                        zsrc = None
                        zoff = b
                    hp = ps.tile([P, W], F32, space="PSUM", name="hp",
                                 tag="hp", padded_shape=[P, DH + 2 * HEADS])
                    for a in range(nA):
                        kk = min(P, Din)
                        ztp = pst.tile([P, P], F32, space="PSUM", name="ztp",
                                       tag="ztp")
                        if norm:
                            src_ap = zsrc[:, a * P:a * P + kk]
                        else:
                            src_ap = zcur[:, b, a * P:a * P + kk]
                        nc.tensor.transpose(out=ztp[:kk, :], in_=src_ap,
                                            identity=ident[:])
                        zts = sp.tile([P, P], F32, name="zts", tag="zts")
                        nc.vector.tensor_copy(out=zts[:kk, :], in_=ztp[:kk, :])
                        nc.tensor.matmul(out=hp[:], lhsT=zts[:kk, :],
                                         rhs=rhs_t[:kk, a, :],
                                         start=(a == 0), stop=(a == nA - 1))
                    pk = sp.tile([P, R], BF, name="pk", tag="pk",
                                 padded_shape=[P, DH + HEADS])
                    nc.vector.tensor_copy(out=pk[:], in_=hp[:, :R])
                    nc.sync.dma_start(
                        out=ag_ins[li][:].rearrange(
                            "(bb p) r -> p bb r", p=P)[:, b, :],
                        in_=pk[:])
                    adt = sp.tile([P, HEADS], F32, name="adt", tag="adt")
                    nc.vector.tensor_copy(out=adt[:, :H], in_=hp[:, R:R + H])
                    nc.sync.dma_start(
                        out=adst_ds[li][:, b * P:(b + 1) * P].rearrange(
                            "h n -> n h"),
                        in_=adt[:, :H])
                nc.gpsimd.collective_compute(
                    "AllGather", mybir.AluOpType.bypass,
                    replica_groups=[list(range(N_CORES))],
                    ins=[ag_ins[li].opt()], outs=[ag_outs[li].opt()])

            def edge_phase(li):
                """Aggregate into zcur[:, :, :D*H] and oden[:, :, :H]."""
                Din, D, H = LAY[li]
                R = D * H + H
                t0 = 0
                for b in range(NB):
                    nchb = int(nch[b])
                    if nchb > 0:
                        idx_t = sp.tile([P, NCHM], I32, name="eix", tag="eix")
                        nc.sync.dma_start(
                            out=idx_t[:, :nchb],
                            in_=t_idx[t0 * P:(t0 + nchb) * P].rearrange(
                                "(p t) -> p t", p=P))
                        dl_t = sp.tile([P, NCHM], F32, name="edt", tag="edt")
                        nc.sync.dma_start(
                            out=dl_t[:, :nchb],
                            in_=t_dl[t0 * P:(t0 + nchb) * P].rearrange(
                                "(p t) -> p t", p=P))
                        g_t = sp.tile([P, NCHM, R], BF, name="g", tag="g",
                                      padded_shape=[P, NCHM, DH + HEADS])
                        for j in range(nchb):
                            nc.gpsimd.indirect_dma_start(
                                out=g_t[:, j, :], out_offset=None,
                                in_=ag_outs[li][:],
                                in_offset=bass.IndirectOffsetOnAxis(
                                    ap=idx_t[:, j, None], axis=0))
                        oh_t = sp.tile([P, NCHM, P], BF, name="oh", tag="oh")
                        nc.vector.tensor_tensor(
                            out=oh_t[:, :nchb, :],
                            in0=iota_t[:, None, :].to_broadcast([P, nchb, P]),
                            in1=dl_t[:, :nchb, None].to_broadcast(
                                [P, nchb, P]),
                            op=mybir.AluOpType.is_equal)
                        ab = sp.tile([P, H, P], F32, name="ab", tag="ab",
                                     padded_shape=[P, HEADS, P])
                        for h in range(H):
                            nc.sync.dma_start(
                                out=ab[:, h, :],
                                in_=adst_ds[li][h, None, b * P:(b + 1) * P
                                                ].to_broadcast([P, P]))
                        v_t = spv.tile([P, NCHM, HEADS, P], F32, name="v",
                                       tag="v")
                        nc.vector.tensor_tensor(
                            out=v_t[:, :nchb, :H, :],
                            in0=oh_t[:, :nchb, None, :].to_broadcast(
                                [P, nchb, H, P]),
                            in1=ab[:, None, :H, :].to_broadcast(
                                [P, nchb, H, P]),
                            op=mybir.AluOpType.mult)
                        et = sp.tile([P, NCHM, HEADS], F32, name="et",
                                     tag="et")
                        nc.vector.tensor_reduce(
                            out=et[:, :nchb, :H], in_=v_t[:, :nchb, :H, :],
                            axis=mybir.AxisListType.X, op=mybir.AluOpType.add)
                        nc.vector.tensor_tensor(
                            out=et[:, :nchb, :H], in0=et[:, :nchb, :H],
                            in1=g_t[:, :nchb, D * H:D * H + H],
                            op=mybir.AluOpType.add)
                        e2 = sp.tile([P, NCHM, HEADS], F32, name="e2",
                                     tag="e2")
                        nc.vector.tensor_scalar(
                            out=e2[:, :nchb, :H], in0=et[:, :nchb, :H],
                            scalar1=NEG_SLOPE, scalar2=None,
                            op0=mybir.AluOpType.mult)
                        nc.vector.tensor_tensor(
                            out=et[:, :nchb, :H], in0=et[:, :nchb, :H],
                            in1=e2[:, :nchb, :H], op=mybir.AluOpType.max)
                        nc.scalar.activation(
                            out=et[:, :nchb, :H], in_=et[:, :nchb, :H],
                            func=mybir.ActivationFunctionType.Exp)
                        p_b = sp.tile([P, NCHM, HEADS], BF, name="pbt",
                                      tag="pbt")
                        nc.vector.tensor_copy(out=p_b[:, :nchb, :H],
                                              in_=et[:, :nchb, :H])
                        m_t = sp.tile([P, NCHM, R], BF, name="m", tag="m",
                                      padded_shape=[P, NCHM, DH + HEADS])
                        nc.vector.tensor_tensor(
                            out=m_t[:, :nchb, :D * H].rearrange(
                                "p t (h d) -> p t h d", h=H),
                            in0=g_t[:, :nchb, :D * H].rearrange(
                                "p t (h d) -> p t h d", h=H),
                            in1=p_b[:, :nchb, :H, None].to_broadcast(
                                [P, nchb, H, D]),
                            op=mybir.AluOpType.mult)
                        nc.vector.tensor_copy(out=m_t[:, :nchb, D * H:],
                                              in_=p_b[:, :nchb, :H])
                        acc = ps.tile([P, R], F32, space="PSUM", name="acc",
                                      tag="hp", padded_shape=[P, DH + 2 * HEADS])
                        for j in range(nchb):
                            nc.tensor.matmul(
                                out=acc[:], lhsT=oh_t[:, j, :],
                                rhs=m_t[:, j, :],
                                start=(j == 0), stop=(j == nchb - 1))
                        nc.vector.tensor_copy(out=zcur[:, b, :D * H],
                                              in_=acc[:, :D * H])
                        nc.vector.tensor_copy(out=oden[:, b, :H],
                                              in_=acc[:, D * H:D * H + H])
                    else:
                        nc.vector.memset(zcur[:, b, :D * H], 0.0)
                        nc.vector.memset(oden[:, b, :H], 0.0)
                    t0 += nchb

            def self_and_div(li):
                Din, D, H = LAY[li]
                R = D * H + H
                for b in range(NB):
                    gsb = sp.tile([P, R], BF, name="gsb", tag="gsb",
                                  padded_shape=[P, DH + HEADS])
                    nc.sync.dma_start(
                        out=gsb[:],
                        in_=ag_ins[li][:].rearrange(
                            "(bb p) r -> p bb r", p=P)[:, b, :])
                    gs = sp.tile([P, R], F32, name="gs", tag="gs",
                                 padded_shape=[P, DH + HEADS])
                    nc.vector.tensor_copy(out=gs[:], in_=gsb[:])
                    ads = sp.tile([P, H], F32, name="ads", tag="ads",
                                  padded_shape=[P, HEADS])
                    nc.sync.dma_start(
                        out=ads[:],
                        in_=adst_ds[li][:, b * P:(b + 1) * P].rearrange(
                            "h n -> n h"))
                    es = sp.tile([P, H], F32, name="es", tag="es",
                                 padded_shape=[P, HEADS])
                    nc.vector.tensor_tensor(out=es[:], in0=gs[:, D * H:],
                                            in1=ads[:],
                                            op=mybir.AluOpType.add)
                    es2 = sp.tile([P, H], F32, name="es2", tag="es2",
                                  padded_shape=[P, HEADS])
                    nc.vector.tensor_scalar(out=es2[:], in0=es[:],
                                            scalar1=NEG_SLOPE, scalar2=None,
                                            op0=mybir.AluOpType.mult)
                    nc.vector.tensor_tensor(out=es[:], in0=es[:], in1=es2[:],
                                            op=mybir.AluOpType.max)
                    nc.scalar.activation(
                        out=es[:], in_=es[:],
                        func=mybir.ActivationFunctionType.Exp)
                    ms = sp.tile([P, D * H], F32, name="ms", tag="ms",
                                 padded_shape=[P, DH])
                    nc.vector.tensor_tensor(
                        out=ms[:].rearrange("p (h d) -> p h d", h=H),
                        in0=gs[:, :D * H].rearrange("p (h d) -> p h d", h=H),
                        in1=es[:, :, None].to_broadcast([P, H, D]),
                        op=mybir.AluOpType.mult)
                    nc.vector.tensor_tensor(out=zcur[:, b, :D * H],
                                            in0=zcur[:, b, :D * H], in1=ms[:],
                                            op=mybir.AluOpType.add)
                    nc.vector.tensor_tensor(out=oden[:, b, :H],
                                            in0=oden[:, b, :H], in1=es[:],
                                            op=mybir.AluOpType.add)
                    # divide
                    nc.vector.tensor_scalar(out=oden[:, b, :H],
                                            in0=oden[:, b, :H], scalar1=1e-16,
                                            scalar2=None,
                                            op0=mybir.AluOpType.add)
                    rc = sp.tile([P, H], F32, name="rc", tag="rc",
                                 padded_shape=[P, HEADS])
                    nc.vector.reciprocal(out=rc[:], in_=oden[:, b, :H])
                    nc.vector.tensor_tensor(
                        out=zcur[:, b, :D * H].rearrange(
                            "p (h d) -> p h d", h=H),
                        in0=zcur[:, b, :D * H].rearrange(
                            "p (h d) -> p h d", h=H),
                        in1=rc[:, :, None].to_broadcast([P, H, D]),
                        op=mybir.AluOpType.mult)

            def bn_phase(Dw):
                """BN over zcur[:, :, :Dw] -> sc_t/sh_t bcast tiles."""
                stat_in = dr.tile([1, 2 * DH], F32, name="statin",
                                  uniquify=True)
                stat_out = dr.tile([1, 2 * DH], F32, addr_space="Shared",
                                   name="statout", uniquify=True)
                ss_d = dr.tile([2, DH], F32, name="ssd", uniquify=True)
                s1 = pss.tile([1, Dw], F32, space="PSUM", name="s1", tag="s1",
                              padded_shape=[1, DH])
                s2 = pss.tile([1, Dw], F32, space="PSUM", name="s2", tag="s2",
                              padded_shape=[1, DH])
                for b in range(NB):
                    ones = ones_f if b < NB - 1 else ones_p
                    nc.tensor.matmul(out=s1[:], lhsT=ones[:],
                                     rhs=zcur[:, b, :Dw],
                                     start=(b == 0), stop=(b == NB - 1))
                for b in range(NB):
                    ones = ones_f if b < NB - 1 else ones_p
                    sq = sp.tile([P, Dw], F32, name="sq", tag="sq",
                                 padded_shape=[P, DH])
                    nc.vector.tensor_tensor(out=sq[:], in0=zcur[:, b, :Dw],
                                            in1=zcur[:, b, :Dw],
                                            op=mybir.AluOpType.mult)
                    nc.tensor.matmul(out=s2[:], lhsT=ones[:], rhs=sq[:],
                                     start=(b == 0), stop=(b == NB - 1))
                srow = sp.tile([1, 2 * DH], F32, name="srow", tag="srow")
                nc.vector.tensor_copy(out=srow[:, :Dw], in_=s1[:])
                nc.vector.tensor_copy(out=srow[:, DH:DH + Dw], in_=s2[:])
                nc.sync.dma_start(out=stat_in[:], in_=srow[:])
                nc.gpsimd.collective_compute(
                    "AllReduce", mybir.AluOpType.add,
                    replica_groups=[list(range(N_CORES))],
                    ins=[stat_in.opt()], outs=[stat_out.opt()])
                gr = sp.tile([1, 2 * DH], F32, name="gr", tag="gr")
                nc.sync.dma_start(out=gr[:], in_=stat_out[:])
                mu = sp.tile([1, DH], F32, name="mu", tag="mu")
                nc.vector.tensor_scalar(out=mu[:, :Dw], in0=gr[:, :Dw],
                                        scalar1=1.0 / N, scalar2=None,
                                        op0=mybir.AluOpType.mult)
                va = sp.tile([1, DH], F32, name="va", tag="va")
                nc.vector.tensor_scalar(out=va[:, :Dw],
                                        in0=gr[:, DH:DH + Dw],
                                        scalar1=1.0 / N, scalar2=None,
                                        op0=mybir.AluOpType.mult)
                m2 = sp.tile([1, DH], F32, name="m2", tag="m2")
                nc.vector.tensor_tensor(out=m2[:, :Dw], in0=mu[:, :Dw],
                                        in1=mu[:, :Dw],
                                        op=mybir.AluOpType.mult)
                nc.vector.tensor_tensor(out=va[:, :Dw], in0=va[:, :Dw],
                                        in1=m2[:, :Dw],
                                        op=mybir.AluOpType.subtract)
                nc.vector.tensor_scalar(out=va[:, :Dw], in0=va[:, :Dw],
                                        scalar1=EPS_BN, scalar2=None,
                                        op0=mybir.AluOpType.add)
                nc.scalar.activation(out=va[:, :Dw], in_=va[:, :Dw],
                                     func=mybir.ActivationFunctionType.Sqrt)
                rcv = sp.tile([1, DH], F32, name="rcv", tag="rcv")
                nc.vector.reciprocal(out=rcv[:, :Dw], in_=va[:, :Dw])
                sh = sp.tile([1, DH], F32, name="sh", tag="sh")
                nc.vector.tensor_tensor(out=sh[:, :Dw], in0=mu[:, :Dw],
                                        in1=rcv[:, :Dw],
                                        op=mybir.AluOpType.mult)
                nc.vector.tensor_scalar(out=sh[:, :Dw], in0=sh[:, :Dw],
                                        scalar1=-1.0, scalar2=None,
                                        op0=mybir.AluOpType.mult)
                nc.sync.dma_start(out=ss_d[0, None, :Dw], in_=rcv[:, :Dw])
                nc.sync.dma_start(out=ss_d[1, None, :Dw], in_=sh[:, :Dw])
                nc.sync.dma_start(out=sc_t[:, :Dw],
                                  in_=ss_d[0, None, :Dw].to_broadcast([P, Dw]))
                nc.sync.dma_start(out=sh_t[:, :Dw],
                                  in_=ss_d[1, None, :Dw].to_broadcast([P, Dw]))

            def mlp_phase():
                lw1_t = cb.tile([HID, HID], F32)
                nc.sync.dma_start(out=lw1_t[:], in_=t_lw1[:])
                lw2_t = cb.tile([HID, 2], F32)
                nc.sync.dma_start(out=lw2_t[:], in_=t_lw2[:])
                # h4 = relu(bn(z2)) @ lw1  (bn applied via sc/sh)
                for b in range(NB):
                    z_t = sp.tile([P, HID], F32, name="z3", tag="z3")
                    nc.vector.tensor_tensor(out=z_t[:], in0=zcur[:, b, :HID],
                                            in1=sc_t[:, :HID],
                                            op=mybir.AluOpType.mult)
                    nc.vector.tensor_tensor(out=z_t[:], in0=z_t[:],
                                            in1=sh_t[:, :HID],
                                            op=mybir.AluOpType.add)
                    nc.vector.tensor_scalar(out=z_t[:], in0=z_t[:],
                                            scalar1=0.0, scalar2=None,
                                            op0=mybir.AluOpType.max)
                    ztp = pst.tile([P, P], F32, space="PSUM", name="ztp4",
                                   tag="ztp")
                    nc.tensor.transpose(out=ztp[:HID, :], in_=z_t[:],
                                        identity=ident[:])
                    zts = sp.tile([P, P], F32, name="zts4", tag="zts")
                    nc.vector.tensor_copy(out=zts[:HID, :], in_=ztp[:HID, :])
                    hp = ps.tile([P, HID], F32, space="PSUM", name="hp4",
                                 tag="hp", padded_shape=[P, DH + 2 * HEADS])
                    nc.tensor.matmul(out=hp[:], lhsT=zts[:HID, :],
                                     rhs=lw1_t[:], start=True, stop=True)
                    nc.vector.tensor_copy(out=zcur[:, b, :HID], in_=hp[:])
                bn_phase(HID)
                for b in range(NB):
                    z_t = sp.tile([P, HID], F32, name="z4", tag="z3")
                    nc.vector.tensor_tensor(out=z_t[:], in0=zcur[:, b, :HID],
                                            in1=sc_t[:, :HID],
                                            op=mybir.AluOpType.mult)
                    nc.vector.tensor_tensor(out=z_t[:], in0=z_t[:],
                                            in1=sh_t[:, :HID],
                                            op=mybir.AluOpType.add)
                    nc.vector.tensor_scalar(out=z_t[:], in0=z_t[:],
                                            scalar1=0.0, scalar2=None,
                                            op0=mybir.AluOpType.max)
                    ztp = pst.tile([P, P], F32, space="PSUM", name="ztp5",
                                   tag="ztp")
                    nc.tensor.transpose(out=ztp[:HID, :], in_=z_t[:],
                                        identity=ident[:])
                    zts = sp.tile([P, P], F32, name="zts5", tag="zts")
                    nc.vector.tensor_copy(out=zts[:HID, :], in_=ztp[:HID, :])
                    op2 = ps.tile([P, 2], F32, space="PSUM", name="op2",
                                  tag="hp", padded_shape=[P, DH + 2 * HEADS])
                    nc.tensor.matmul(out=op2[:], lhsT=zts[:HID, :],
                                     rhs=lw2_t[:], start=True, stop=True)
                    ot = sp.tile([P, 2], F32, name="ot", tag="ot")
                    nc.vector.tensor_tensor(out=ot[:], in0=op2[:],
                                            in1=lb2_b[:],
                                            op=mybir.AluOpType.add)
                    nc.sync.dma_start(
                        out=t_out[:].rearrange("(bb p) c -> p bb c",
                                               p=P)[:, b, :],
                        in_=ot[:])

            def dump(li):
                if not DEBUG_DUMP:
                    return
                D = LAY[li][1] * LAY[li][2]
                for b in range(NB):
                    dt_ = sp.tile([P, DH + HEADS], F32, name="dmp", tag="dmp")
                    nc.vector.tensor_copy(out=dt_[:, :D], in_=zcur[:, b, :D])
                    nc.vector.tensor_copy(out=dt_[:, D:D + HEADS],
                                          in_=oden[:, b, :])
                    nc.sync.dma_start(
                        out=t_dbg[li][:].rearrange("(bb p) r -> p bb r",
                                                   p=P)[:, b, :],
                        in_=dt_[:, :D + HEADS])

            # ---------------- driver ----------------
            dense_phase(0, norm=False, Dprev=F_IN)
            edge_phase(0)
            self_and_div(0)
            dump(0)
            bn_phase(DH)
            dense_phase(1, norm=True, Dprev=DH)
            edge_phase(1)
            self_and_div(1)
            dump(1)
            bn_phase(HID)
            dense_phase(2, norm=True, Dprev=HID)
            edge_phase(2)
            self_and_div(2)
            dump(2)
            bn_phase(HID)
            mlp_phase()

    nc.compile()
    return nc


def _fuse_weights(W, a_s, a_d, H, D):
    """rhs = [W | W@a_s per head | W@a_d per head]  -> [Din, H*D + 2H]."""
    W = np.asarray(W, np.float32)
    was = np.stack([W[:, h * D:(h + 1) * D] @ np.asarray(a_s, np.float32)[h]
                    for h in range(H)], axis=1)
    wad = np.stack([W[:, h * D:(h + 1) * D] @ np.asarray(a_d, np.float32)[h]
                    for h in range(H)], axis=1)
    return np.concatenate([W, was, wad], axis=1)


def kernel(x, edge_index, W0, as0, ad0, b0, W1, as1, ad1, b1,
           W2, as2, ad2, b2, lw1, lb1, lw2, lb2):
    from concourse.bass_utils import run_bass_kernel_spmd

    NL, NB, NLP, LAST = _dims()
    x = np.ascontiguousarray(np.asarray(x, np.float32))
    nch, idx_bufs, dl_bufs = _host_prep_edges(np.asarray(edge_index))

    key = ("gat", tuple(nch.tolist()))
    if key not in _STATE:
        _STATE.clear()
        _STATE[key] = _build(nch)
    nc = _STATE[key]

    rhs0 = _fuse_weights(W0, as0, ad0, HEADS, HID)
    rhs1 = _fuse_weights(W1, as1, ad1, 1, HID)
    rhs2 = _fuse_weights(W2, as2, ad2, 1, HID)
    iota = np.broadcast_to(np.arange(P, dtype=np.float32), (P, P)).copy()

    in_maps = []
    for c in range(N_CORES):
        xs = np.zeros((NLP, F_IN), np.float32)
        xs[:NL] = x[c * NL:(c + 1) * NL]
        in_maps.append({
            "x": xs,
            "rhs0": rhs0, "rhs1": rhs1, "rhs2": rhs2,
            "lw1": np.asarray(lw1, np.float32),
            "lw2": np.asarray(lw2, np.float32),
            "lb2": np.asarray(lb2, np.float32).reshape(1, 2),
            "iota": iota,
            "eidx": idx_bufs[c], "edl": dl_bufs[c],
        })
    res = run_bass_kernel_spmd(nc, in_maps, list(range(N_CORES)))
    out = np.concatenate([res.results[c]["out"][:NL] for c in range(N_CORES)],
                         axis=0)
    if DEBUG_DUMP:
        global _DBG
        _DBG = [np.concatenate([res.results[c][f"dbg{li}"][:NL]
                                for c in range(N_CORES)], axis=0)
                for li in range(3)]
    return out.astype(np.float32)
